# revision 28
# baseline (speedup 1.0000x reference)
"""GQA (16 q-heads / 4 kv-heads, D=128, S=2048, E=2048, B=2) on 8 trn2 cores.

Sharding: core = 4*b + g  (b in {0,1} batch, g in {0..3} kv-head group).
Each core computes its batch's 4 query heads (one kv group) end-to-end and
the host sums the 4 partial o_proj outputs per batch.

v5 (Act/PE balanced, ping-ponged heads, ledger-scheduled fills):
  - Host pre-arranges every tensor partition-major so each DMA moves >=512B
    contiguous runs; wq/wk/wv/wo stay resident in SBUF as fp8 hi/lo
    double-quant planes; x resident per 512-position chunk.
  - Projections are fp8 DoubleRow over 3 plane products (HH+HL+LH); V is
    projected directly into natural [keys, D] layout (x-tile stationary),
    removing the PE transpose pass.
  - Attention in four 512-wide query chunks; key tiles in PAIRS: scores for
    tiles 2j,2j+1 land in one [128,2,512] PSUM tile and take ONE Act exp
    instruction.  TWO heads are ping-ponged per pass so the sc-PSUM reuse
    dependency always has a full other-head iteration of cover; both heads
    share one [128,2,512] AV accumulator.
  - Non-attention work (remaining K/V chunks, Q projections, o_proj) is
    queued as ~2-matmul fill units drained a couple per pair-iteration; a
    label ledger (require()) force-drains producers before any consumer is
    emitted, so fills can flow across window boundaries safely.
  - o_proj accumulates in PSUM, stages bf16 through SBUF ([128,1024] tiles,
    copies alternating Act/DVE), stores direct; host sums partials in f32.
"""

import numpy as np
import ml_dtypes

import concourse.bass as bass
import concourse.bacc as bacc
import concourse.mybir as mybir
import concourse.tile as tile
from concourse.bass_utils import run_bass_kernel_spmd

B, S, E = 2, 2048, 2048
H, HKV, D = 16, 4, 128
G = H // HKV          # 4 query heads per kv group
GD = G * D            # 512 channels per group
NCORES = 8
SCALE = 1.0 / float(np.sqrt(D))
ROPE_BASE = 10000.0
AX = 16.0             # fp8 plane scale for x
AW = 64.0             # fp8 plane scale for wq/wk/wv/wo
PSC = AX * AW         # q/k/v come out scaled by PSC
SCALE_EFF = SCALE / (PSC * PSC)   # folds the q*k scale into exp
AO = 16.0             # fp8 plane scale for the normalized attention output
# the softmax reduce uses (PSC/AO)-valued "ones", so ot = AO * attn_out and
# the o_proj result comes out scaled by AO*AW = PSC; the host divides once.
RED = PSC / AO

NE = E // 128         # 16 e-blocks (contraction for projections)
NC4 = S // 512        # 4 position chunks of 512
NST = S // 128        # 16 sk-tiles of 128
NP = NST // 2         # 8 sk-tile PAIRS

F32 = mybir.dt.float32
BF16 = mybir.dt.bfloat16
FP8 = mybir.dt.float8e4
DR = mybir.MatmulPerfMode.DoubleRow
AF = mybir.ActivationFunctionType
OP = mybir.AluOpType

PLANES = ((0, 0), (0, 1), (1, 0))   # (w_plane, x_plane): HH, HL, LH


def _emit(nc, tc, xh, xl, wqh, wql, wkh, wkl, wvh, wvl, woh, wol, cosT,
          sinT, rotP, onesb, out):
    from contextlib import ExitStack
    import collections
    es = ExitStack()
    with es:
        cpool = es.enter_context(tc.tile_pool(name="const", bufs=1))
        xpool = es.enter_context(tc.tile_pool(name="xs", bufs=1))
        rpool = es.enter_context(tc.tile_pool(name="rope", bufs=2))
        etpool = es.enter_context(tc.tile_pool(name="et", bufs=6))
        bcspool = es.enter_context(tc.tile_pool(name="bcs", bufs=2))
        dnpool = es.enter_context(tc.tile_pool(name="dn", bufs=2))
        rcpool = es.enter_context(tc.tile_pool(name="rc", bufs=2))
        otpool = es.enter_context(tc.tile_pool(name="ot", bufs=2))
        ostgpool = es.enter_context(tc.tile_pool(name="ostg", bufs=6))
        pssc = es.enter_context(
            tc.tile_pool(name="pssc", bufs=2, space=bass.MemorySpace.PSUM))
        psav = es.enter_context(
            tc.tile_pool(name="psav", bufs=1, space=bass.MemorySpace.PSUM))
        psmx = es.enter_context(
            tc.tile_pool(name="psmx", bufs=2, space=bass.MemorySpace.PSUM))

        # ---- persistent SBUF tensors ----
        rp_sb = cpool.tile([128, 128], BF16, tag="rp")
        ones_sb = cpool.tile([128, 1], BF16, tag="ones")
        cos_sb = cpool.tile([D, S], BF16, tag="cos")
        sin_sb = cpool.tile([D, S], BF16, tag="sin")
        wk_t = [cpool.tile([128, NE, D], FP8, tag=f"wkt{i}", name=f"wkt{i}")
                for i in range(2)]
        wv_t = [cpool.tile([128, NE, D], FP8, tag=f"wvt{i}", name=f"wvt{i}")
                for i in range(2)]
        wq_t = [cpool.tile([128, NE, GD], FP8, tag=f"wqt{i}", name=f"wqt{i}")
                for i in range(2)]
        wo_t = [cpool.tile([128, G, E], FP8, tag=f"wot{i}", name=f"wot{i}")
                for i in range(2)]
        onescol = cpool.tile([1, 128], BF16, tag="onescol")
        kt = cpool.tile([D, S], BF16, tag="kt")
        qt = [cpool.tile([D, S], BF16, tag=f"qt{h}", name=f"qt{h}")
              for h in range(G)]
        vn = cpool.tile([128, NST, D], BF16, tag="vn")

        xt = {}

        def load_x(c4, nsplit):
            for i, t in enumerate((xh, xl)):
                xtile = xpool.tile([128, NE, 512], FP8, tag=f"x{c4}_{i}",
                                   name=f"x{c4}_{i}")
                step = NE // nsplit
                for s in range(nsplit):
                    nc.sync.dma_start(
                        out=xtile[:, s * step:(s + 1) * step, :],
                        in_=t.ap()[:, s * step:(s + 1) * step,
                                   c4 * 512:(c4 + 1) * 512])
                xt[(c4, i)] = xtile

        # ---- DMA schedule (dependency order; contiguous runs >=512B).
        # Order matches the fused K0+Q0 phase-A plane order HH, LH, HL:
        # wk/wq hi planes + xh first, then lo weight planes, then xl.
        nc.sync.dma_start(out=wk_t[0][:], in_=wkh.ap())
        for sp in range(2):
            nc.sync.dma_start(out=wq_t[0][:, sp * 8:(sp + 1) * 8, :],
                              in_=wqh.ap()[:, sp * 8:(sp + 1) * 8, :])
        xt0h = xpool.tile([128, NE, 512], FP8, tag="x0_0", name="x0_0")
        xt0l = xpool.tile([128, NE, 512], FP8, tag="x0_1", name="x0_1")
        xt[(0, 0)], xt[(0, 1)] = xt0h, xt0l
        for sp in range(4):
            nc.sync.dma_start(out=xt0h[:, sp * 4:(sp + 1) * 4, :],
                              in_=xh.ap()[:, sp * 4:(sp + 1) * 4, 0:512])
        nc.sync.dma_start(out=wk_t[1][:], in_=wkl.ap())
        for sp in range(2):
            nc.sync.dma_start(out=wq_t[1][:, sp * 8:(sp + 1) * 8, :],
                              in_=wql.ap()[:, sp * 8:(sp + 1) * 8, :])
        for sp in range(4):
            nc.sync.dma_start(out=xt0l[:, sp * 4:(sp + 1) * 4, :],
                              in_=xl.ap()[:, sp * 4:(sp + 1) * 4, 0:512])

        def load_cs(c4):
            csl = slice(c4 * 512, (c4 + 1) * 512)
            nc.sync.dma_start(out=cos_sb[:, csl], in_=cosT.ap()[:, csl])
            nc.sync.dma_start(out=sin_sb[:, csl], in_=sinT.ap()[:, csl])

        load_cs(0)
        nc.sync.dma_start(out=rp_sb[:], in_=rotP.ap())
        nc.sync.dma_start(out=ones_sb[:], in_=onesb.ap())
        nc.vector.memset(onescol[:], 1.0)
        for i, t in enumerate((wvh, wvl)):
            nc.sync.dma_start(out=wv_t[i][:], in_=t.ap())
        load_x(1, 2)
        load_cs(1)
        load_x(2, 2)
        load_cs(2)
        load_x(3, 2)
        load_cs(3)
        for i, t in enumerate((woh, wol)):
            for sp in range(2):
                nc.sync.dma_start(
                    out=wo_t[i][:, sp * 2:(sp + 1) * 2, :],
                    in_=t.ap()[:, sp * 2:(sp + 1) * 2, :])

        # ---- fill-unit queues (labelled; require() force-drains FIFO
        # until a label's units are all emitted -- keeps emission order
        # consistent with data dependencies) ----
        fill = collections.deque()
        tailq = collections.deque()
        pending = collections.Counter()

        def enq(label, fn, q=None):
            (fill if q is None else q).append((label, fn))
            pending[label] += 1

        def drain(n, q=None):
            q = fill if q is None else q
            while n > 0 and q:
                lab, fn = q.popleft()
                fn()
                pending[lab] -= 1
                n -= 1

        def require(label):
            while pending.get(label, 0) > 0:
                lab, fn = fill.popleft()
                fn()
                pending[lab] -= 1

        # ---- rope: rotate_half as signed-permutation matmul ----
        def rope_start(ps, eng):
            qraw = rpool.tile([128, 512], BF16, tag="qraw")
            if eng == 'act':
                nc.scalar.copy(qraw[:], ps[:])
            else:
                nc.vector.tensor_copy(qraw[:], ps[:])
            return qraw

        def rope_finish(dst, qraw, sl):
            tmc = rpool.tile([128, 512], BF16, tag="tmc")
            t2 = rpool.tile([128, 512], BF16, tag="t2")
            rot = psmx.tile([128, 512], F32, tag="mx", name="rot")
            nc.tensor.matmul(rot[:], rp_sb[:], qraw[:], start=True, stop=True)
            nc.gpsimd.tensor_tensor(tmc[:], qraw[:], cos_sb[:, sl], OP.mult)
            nc.vector.tensor_tensor(t2[:], rot[:], sin_sb[:, sl], OP.mult)
            nc.vector.tensor_tensor(dst, tmc[:], t2[:], OP.add)

        # ---- projections (fp8 DoubleRow, 3 quant planes) ----
        def proj_mms(wt, cslice, c4):
            mms = []
            for wi, xi in PLANES:
                for p in range(NE // 2):
                    mms.append((wt[wi][:, 2 * p:2 * p + 2, cslice],
                                xt[(c4, xi)][:, 2 * p:2 * p + 2, :]))
            return mms

        def kproj(c4):
            sl = slice(c4 * 512, (c4 + 1) * 512)
            ps = psmx.tile([128, 512], F32, tag="mx", name="ps")
            mms = proj_mms(wk_t, slice(0, D), c4)
            for i, (wa, xa) in enumerate(mms):
                nc.tensor.matmul(ps[:], wa, xa, perf_mode=DR,
                                 start=(i == 0), stop=(i == len(mms) - 1))
            return ps, sl

        def vproj_mms(c4):
            vp = psmx.tile([128, 4, 128], F32, tag="mx", name="vp")
            for i in range(4):
                ksl = slice(i * 128, (i + 1) * 128)
                j = 0
                for wi, xi in PLANES:
                    for p in range(NE // 2):
                        nc.tensor.matmul(
                            vp[:, i, :],
                            xt[(c4, xi)][:, 2 * p:2 * p + 2, ksl],
                            wv_t[wi][:, 2 * p:2 * p + 2, :],
                            perf_mode=DR, start=(j == 0), stop=(j == 23))
                        j += 1
            return vp

        def vn_copy(c4, vp, eng):
            dst = vn[:, c4 * 4:(c4 + 1) * 4, :]
            if eng == 'act':
                nc.scalar.copy(dst, vp[:])
            elif eng == 'pool':
                nc.gpsimd.tensor_copy(dst, vp[:])
            else:
                nc.vector.tensor_copy(dst, vp[:])

        def qproj(h, c4):
            # phase-A inline Q projection; rope finished by caller interleave
            sl = slice(c4 * 512, (c4 + 1) * 512)
            ps = psmx.tile([128, 512], F32, tag="mx", name="ps")
            mms = proj_mms(wq_t, slice(h * D, (h + 1) * D), c4)
            for i, (wa, xa) in enumerate(mms):
                nc.tensor.matmul(ps[:], wa, xa, perf_mode=DR,
                                 start=(i == 0), stop=(i == len(mms) - 1))
            return ps, sl

        def enqueue_qproj(h, c4):
            sl = slice(c4 * 512, (c4 + 1) * 512)
            lab = f"Q{c4}h{h}"
            state = {}
            nmm = 24

            def mk(j):
                def unit():
                    if j == 0:
                        state['ps'] = psmx.tile([128, 512], F32, tag="mx",
                                                name="ps")
                        state['mms'] = proj_mms(
                            wq_t, slice(h * D, (h + 1) * D), c4)
                    ps = state['ps']
                    for jj in (2 * j, 2 * j + 1):
                        wa, xa = state['mms'][jj]
                        nc.tensor.matmul(ps[:], wa, xa, perf_mode=DR,
                                         start=(jj == 0), stop=(jj == nmm - 1))
                return unit
            for j in range(nmm // 2):
                enq(lab, mk(j))

            def fin():
                qraw = rope_start(state['ps'], 'dve')
                drain(2)   # cover the DVE copy latency with queued PE work
                rope_finish(qt[h][:, sl], qraw, sl)
            enq(lab, fin)

        def enqueue_kproj(c4):
            sl = slice(c4 * 512, (c4 + 1) * 512)
            lab = f"K{c4}"
            state = {}
            nmm = 24

            def mk(j):
                def unit():
                    if j == 0:
                        state['ps'] = psmx.tile([128, 512], F32, tag="mx",
                                                name="ps")
                        state['mms'] = proj_mms(wk_t, slice(0, D), c4)
                    ps = state['ps']
                    for jj in (2 * j, 2 * j + 1):
                        wa, xa = state['mms'][jj]
                        nc.tensor.matmul(ps[:], wa, xa, perf_mode=DR,
                                         start=(jj == 0), stop=(jj == nmm - 1))
                return unit
            for j in range(nmm // 2):
                enq(lab, mk(j))

            def fin():
                qraw = rope_start(state['ps'], 'dve')
                drain(2)   # cover the DVE copy latency with queued PE work
                rope_finish(kt[:, sl], qraw, sl)
            enq(lab, fin)

        def enqueue_vproj(c4):
            lab = f"V{c4}"
            state = {}

            def mkmm(i, g):
                def unit():
                    if i == 0 and g == 0:
                        state['vp'] = psmx.tile([128, 4, 128], F32, tag="mx",
                                                name="vp")
                    vp = state['vp']
                    ksl = slice(i * 128, (i + 1) * 128)
                    mms = [(xt[(c4, xi)][:, 2 * p:2 * p + 2, ksl],
                            wv_t[wi][:, 2 * p:2 * p + 2, :])
                           for wi, xi in PLANES for p in range(NE // 2)]
                    for jj in range(8 * g, 8 * g + 8):
                        sa, ma = mms[jj]
                        nc.tensor.matmul(vp[:, i, :], sa, ma, perf_mode=DR,
                                         start=(jj == 0), stop=(jj == 23))
                return unit

            def mkcp(i):
                def unit():
                    nc.vector.tensor_copy(vn[:, c4 * 4 + i, :],
                                          state['vp'][:, i, :])
                return unit
            for i in range(4):
                for g in range(3):
                    enq(lab, mkmm(i, g))
                enq(lab, mkcp(i))

        # ---- o_proj: ot (fp8 hi/lo planes) @ wo, PSUM -> bf16 SBUF
        # staging (two 512-col groups share one [128,1024] staging tile and
        # one store) ----
        def enqueue_oproj(ci, oth, otl, q):
            off = ci * 512
            for st in range(4):
                ssl = slice(st * 128, (st + 1) * 128)
                shared = {}
                for eo in range(4):
                    esl = slice(eo * 512, (eo + 1) * 512)
                    state = {}
                    # head-pair i=0 mms first: they only need heads 0/1 of
                    # the ot planes, so the tail can start before the last
                    # head's epilogue lands.
                    mms = []
                    for i in range(2):
                        for src, wi in ((oth, 0), (oth, 1), (otl, 0)):
                            mms.append((src[:, 2 * i:2 * i + 2, ssl],
                                        wo_t[wi][:, 2 * i:2 * i + 2, esl]))

                    def mk(st, eo, j, mms=mms, state=state, shared=shared):
                        def unit():
                            if j == 0:
                                state['op'] = psmx.tile([128, 512], F32,
                                                        tag="mx", name="op")
                            op = state['op']
                            for jj in (2 * j, 2 * j + 1):
                                oa, wa = mms[jj]
                                nc.tensor.matmul(op[:], oa, wa, perf_mode=DR,
                                                 start=(jj == 0),
                                                 stop=(jj == 5))
                            if j == 2:
                                single = q is tailq and st == 3 and eo >= 2
                                half = eo % 2
                                if half == 0 or single:
                                    shared['ostg'] = ostgpool.tile(
                                        [128, 1024], BF16, tag="ostg",
                                        name="ostg")
                                ostg = shared['ostg']
                                dst = ostg[:, half * 512:(half + 1) * 512]
                                # GPSIMD cannot read PSUM: alternate Act/DVE
                                if (st + eo) % 2 == 0 if single else (
                                        st + eo // 2) % 2 == 0:
                                    nc.scalar.copy(dst, op[:])
                                else:
                                    nc.vector.tensor_copy(dst, op[:])
                                if single:
                                    nc.sync.dma_start(
                                        out=out.ap()[off + st * 128:
                                                     off + (st + 1) * 128,
                                                     eo * 512:
                                                     (eo + 1) * 512],
                                        in_=dst)
                                elif half == 1:
                                    nc.sync.dma_start(
                                        out=out.ap()[off + st * 128:
                                                     off + (st + 1) * 128,
                                                     (eo - 1) * 512:
                                                     (eo + 1) * 512],
                                        in_=ostg[:])
                        return unit
                    for j in range(3):
                        enq(f"O{ci}", mk(st, eo, j), q)

        # ---- K/V as fill units (not used in final schedule; kept simple) --

        # ---- attention: paired key tiles, TWO heads ping-ponged so the
        # sc-PSUM reuse dependency (exp of the same head, 2 pairs back)
        # always has a full other-head iteration of cover; Act runs
        # back-to-back exps and PE never waits on the 2-buffer sc pool ----
        def attn_pass(ci, ha, hb, oth, otl, drain_n):
            off = ci * 512
            qsl = slice(off, off + 512)
            require(f"Q{ci}h{ha}")
            require(f"Q{ci}h{hb}")
            dnA = dnpool.tile([128, 512], BF16, tag="dn", name="dnA")
            dnB = dnpool.tile([128, 512], BF16, tag="dn", name="dnB")
            av2 = psav.tile([128, 2, 512], F32, tag="av")

            def scp_exp(h, j):
                require(f"K{(2 * j + 1) // 4}")
                sc = pssc.tile([128, 2, 512], F32, tag="sc")
                for tt in range(2):
                    t = 2 * j + tt
                    nc.tensor.matmul(sc[:, tt, :],
                                     kt[:, t * 128:(t + 1) * 128],
                                     qt[h][:, qsl], start=True, stop=True)
                et = etpool.tile([128, 2, 512], BF16, tag="et")
                nc.scalar.activation(et[:], sc[:], AF.Exp, scale=SCALE_EFF)
                return et

            def avp(sl_i, j, et):
                require(f"V{(2 * j + 1) // 4}")
                for tt in range(2):
                    t = 2 * j + tt
                    nc.tensor.matmul(av2[:, sl_i, :], vn[:, t, :],
                                     et[:, tt, :],
                                     start=(t == 0), stop=(t == NST - 1))

            def dnp(dn, j, et):
                for tt in range(2):
                    if j == 0 and tt == 0:
                        nc.vector.tensor_copy(dn[:], et[:, 0, :])
                    else:
                        nc.vector.tensor_tensor(dn[:], dn[:], et[:, tt, :],
                                                OP.add)

            def epilogue(h, sl_i, dn):
                drain(6)
                sm = psmx.tile([1, 512], F32, tag="mx", name="sm")
                nc.tensor.matmul(sm[:], ones_sb[:, 0:1], dn[:],
                                 start=True, stop=True)
                rc = rcpool.tile([1, 512], BF16, tag="rc")
                with nc.allow_low_precision(reason="bf16 denom recip"):
                    nc.vector.reciprocal(rc[:], sm[:])
                otf = rpool.tile([D, 512], F32, tag="otf")
                bcs = bcspool.tile([128, 512], BF16, tag="bcs")
                nc.gpsimd.partition_broadcast(bcs[:], rc[:])
                nc.vector.tensor_tensor(otf[:], av2[:, sl_i, :], bcs[:],
                                        OP.mult)
                if h >= 2:
                    nc.scalar.copy(oth[:, h, :], otf[:])
                else:
                    nc.gpsimd.tensor_copy(oth[:, h, :], otf[:])
                nc.vector.tensor_tensor(otl[:, h, :], otf[:], oth[:, h, :],
                                        OP.subtract)

            etsA = {}
            etsB = {}
            etsA[0] = scp_exp(ha, 0)
            etsB[0] = scp_exp(hb, 0)
            etsA[1] = scp_exp(ha, 1)
            drain(drain_n)
            etsB[1] = scp_exp(hb, 1)
            drain(max(drain_n - 1, 1))
            for j in range(2, NP):
                etsA[j] = scp_exp(ha, j)
                drain(drain_n if j % 2 else max(drain_n - 1, 1))
                avp(0, j - 2, etsA[j - 2])
                dnp(dnA, j - 2, etsA[j - 2])
                del etsA[j - 2]
                etsB[j] = scp_exp(hb, j)
                drain(max(drain_n - 1, 1) if j % 2 else drain_n)
                avp(1, j - 2, etsB[j - 2])
                dnp(dnB, j - 2, etsB[j - 2])
                del etsB[j - 2]
            drain(drain_n)
            avp(0, NP - 2, etsA[NP - 2])
            dnp(dnA, NP - 2, etsA[NP - 2])
            avp(1, NP - 2, etsB[NP - 2])
            dnp(dnB, NP - 2, etsB[NP - 2])
            drain(drain_n)
            avp(0, NP - 1, etsA[NP - 1])
            dnp(dnA, NP - 1, etsA[NP - 1])
            epilogue(ha, 0, dnA)
            avp(1, NP - 1, etsB[NP - 1])
            dnp(dnB, NP - 1, etsB[NP - 1])
            epilogue(hb, 1, dnB)

        # ====== phase A: K0 and Q0 (all 4 heads) interleaved per x-pair,
        # each accumulating in its own PSUM bank (borrowing the idle
        # pssc/psav pools); then V0; K1/V1 and the rest drain into B0 ======
        psK = psmx.tile([128, 512], F32, tag="mx", name="psK")
        psQ3 = psmx.tile([128, 512], F32, tag="mx", name="psQ3")
        psQ0 = pssc.tile([128, 512], F32, tag="sc", name="psQ0")
        psQ1 = pssc.tile([128, 512], F32, tag="sc", name="psQ1")
        psQ2 = psav.tile([128, 512], F32, tag="av", name="psQ2")
        psQ = [psQ0, psQ1, psQ2, psQ3]
        PL_A = ((0, 0), (1, 0), (0, 1))   # HH, LH, HL: xl consumed last
        kj = 0
        qj = [0, 0, 0, 0]
        for wi, xi in PL_A:
            for p in range(NE // 2):
                nc.tensor.matmul(psK[:], wk_t[wi][:, 2 * p:2 * p + 2, 0:D],
                                 xt[(0, xi)][:, 2 * p:2 * p + 2, :],
                                 perf_mode=DR, start=(kj == 0),
                                 stop=(kj == 23))
                kj += 1
                for h in range(G):
                    nc.tensor.matmul(
                        psQ[h][:],
                        wq_t[wi][:, 2 * p:2 * p + 2, h * D:(h + 1) * D],
                        xt[(0, xi)][:, 2 * p:2 * p + 2, :],
                        perf_mode=DR, start=(qj[h] == 0), stop=(qj[h] == 23))
                    qj[h] += 1
        sl0 = slice(0, 512)
        # borrow-frees must be emitted before the pools are re-allocated
        qrawK = rope_start(psK, 'act')
        qraw3 = rope_start(psQ3, 'act')
        vp = vproj_mms(0)
        rope_finish(kt[:, sl0], qrawK, sl0)
        vn_copy(0, vp, 'act')
        qraw0 = rope_start(psQ0, 'dve')
        rope_finish(qt[3][:, sl0], qraw3, sl0)
        qraw1 = rope_start(psQ1, 'dve')
        rope_finish(qt[0][:, sl0], qraw0, sl0)
        qraw2 = rope_start(psQ2, 'dve')
        rope_finish(qt[1][:, sl0], qraw1, sl0)
        rope_finish(qt[2][:, sl0], qraw2, sl0)

        # ================= B windows: attention + drained fills =============
        def ot_planes(ci):
            hi = otpool.tile([128, G, 512], FP8, tag="oth", name=f"oth{ci}")
            lo = otpool.tile([128, G, 512], FP8, tag="otl", name=f"otl{ci}")
            return hi, lo

        planes = {}
        # B0: fills = K/V chunks 2,3 (ledger-paced), then Q chunk 1
        planes[0] = ot_planes(0)
        enqueue_kproj(1)
        enqueue_vproj(1)
        enqueue_kproj(2)
        enqueue_vproj(2)
        enqueue_kproj(3)
        enqueue_vproj(3)
        for hq in range(G):
            enqueue_qproj(hq, 1)
        attn_pass(0, 0, 1, planes[0][0], planes[0][1], 2)
        attn_pass(0, 2, 3, planes[0][0], planes[0][1], 2)
        # B1: fills += Q chunk 2 + o_proj of chunk 0
        planes[1] = ot_planes(1)
        for hq in range(G):
            enqueue_qproj(hq, 2)
        attn_pass(1, 0, 1, planes[1][0], planes[1][1], 2)
        enqueue_oproj(0, planes[0][0], planes[0][1], fill)
        attn_pass(1, 2, 3, planes[1][0], planes[1][1], 2)
        # B2: fills += Q chunk 3 + o_proj of chunk 1
        planes[2] = ot_planes(2)
        for hq in range(G):
            enqueue_qproj(hq, 3)
        attn_pass(2, 0, 1, planes[2][0], planes[2][1], 2)
        enqueue_oproj(1, planes[1][0], planes[1][1], fill)
        attn_pass(2, 2, 3, planes[2][0], planes[2][1], 2)
        # B3: fills += o_proj of chunk 2
        planes[3] = ot_planes(3)
        attn_pass(3, 0, 1, planes[3][0], planes[3][1], 2)
        enqueue_oproj(2, planes[2][0], planes[2][1], fill)
        attn_pass(3, 2, 3, planes[3][0], planes[3][1], 2)
        drain(len(fill))
        # tail: o_proj of chunk 3
        enqueue_oproj(3, planes[3][0], planes[3][1], tailq)
        drain(len(tailq), tailq)


def _build():
    nc = bacc.Bacc("TRN2", target_bir_lowering=False, debug=False,
                   num_devices=NCORES)
    xh = nc.dram_tensor("xh", [128, NE, S], FP8, kind="ExternalInput")
    xl = nc.dram_tensor("xl", [128, NE, S], FP8, kind="ExternalInput")
    wqh = nc.dram_tensor("wqh", [128, NE, GD], FP8, kind="ExternalInput")
    wql = nc.dram_tensor("wql", [128, NE, GD], FP8, kind="ExternalInput")
    wkh = nc.dram_tensor("wkh", [128, NE, D], FP8, kind="ExternalInput")
    wkl = nc.dram_tensor("wkl", [128, NE, D], FP8, kind="ExternalInput")
    wvh = nc.dram_tensor("wvh", [128, NE, D], FP8, kind="ExternalInput")
    wvl = nc.dram_tensor("wvl", [128, NE, D], FP8, kind="ExternalInput")
    woh = nc.dram_tensor("woh", [128, G, E], FP8, kind="ExternalInput")
    wol = nc.dram_tensor("wol", [128, G, E], FP8, kind="ExternalInput")
    cosT = nc.dram_tensor("cosT", [D, S], BF16, kind="ExternalInput")
    sinT = nc.dram_tensor("sinT", [D, S], BF16, kind="ExternalInput")
    rotP = nc.dram_tensor("rotP", [128, 128], BF16, kind="ExternalInput")
    onesb = nc.dram_tensor("onesb", [128, 1], BF16, kind="ExternalInput")
    out = nc.dram_tensor("out", [S, E], BF16, kind="ExternalOutput")
    with tile.TileContext(nc) as tc:
        _emit(nc, tc, xh, xl, wqh, wql, wkh, wkl, wvh, wvl, woh, wol, cosT,
              sinT, rotP, onesb, out)
    nc.compile()
    return nc


def _rope_tables():
    inv = 1.0 / (ROPE_BASE ** (np.arange(0, D, 2, dtype=np.float64) / D))
    t = np.arange(S, dtype=np.float64)
    freqs = t[:, None] * inv[None, :]                    # [S, D/2]
    emb = np.concatenate([freqs, freqs], axis=-1)        # [S, D]
    cosT = np.cos(emb).T.astype(ml_dtypes.bfloat16)      # [D, S]
    sinT = np.sin(emb).T.astype(ml_dtypes.bfloat16)
    return np.ascontiguousarray(cosT), np.ascontiguousarray(sinT)


def _rot_perm():
    # rot(q)[d] = -q[d+64] for d<64, +q[d-64] for d>=64, as a stationary
    # matmul operand: rot = P^T @ q with P[k, m] below.
    p = np.zeros((128, 128), dtype=ml_dtypes.bfloat16)
    for d in range(64):
        p[d + 64, d] = -1.0
        p[d, d + 64] = 1.0
    return p


def _pm(a, nblk):
    """[K, M] -> partition-major [128, nblk, M] (K = nblk*128)."""
    k, m = a.shape
    return np.ascontiguousarray(a.reshape(nblk, 128, m).transpose(1, 0, 2))


_NC = None
LAST_RESULTS = None


def kernel(hidden_states, wq, wk, wv, wo):
    global _NC, LAST_RESULTS
    if _NC is None:
        _NC = _build()
    cosT, sinT = _rope_tables()
    onesb = np.full((128, 1), RED, dtype=ml_dtypes.bfloat16)
    rotP = _rot_perm()
    f8 = ml_dtypes.float8_e4m3

    def planes(a, scale):
        hi = (scale * a).astype(f8)
        lo = (scale * a - hi.astype(np.float32)).astype(f8)
        return hi, lo

    hs = np.asarray(hidden_states, dtype=np.float32)
    wq = np.asarray(wq, dtype=np.float32)
    wk = np.asarray(wk, dtype=np.float32)
    wv = np.asarray(wv, dtype=np.float32)
    wo = np.asarray(wo, dtype=np.float32)
    xplanes = []
    for b in range(B):
        hi, lo = planes(np.ascontiguousarray(hs[b].T), AX)
        xplanes.append((_pm(hi, NE), _pm(lo, NE)))

    in_maps = []
    for core in range(NCORES):
        b, g = divmod(core, G)
        wqh_, wql_ = planes(wq[:, GD * g:GD * (g + 1)], AW)
        wkh_, wkl_ = planes(wk[:, D * g:D * (g + 1)], AW)
        wvh_, wvl_ = planes(wv[:, D * g:D * (g + 1)], AW)
        woh_, wol_ = planes(wo[GD * g:GD * (g + 1), :], AW)
        in_maps.append({
            "xh": xplanes[b][0],
            "xl": xplanes[b][1],
            "wqh": _pm(wqh_, NE),
            "wql": _pm(wql_, NE),
            "wkh": _pm(wkh_, NE),
            "wkl": _pm(wkl_, NE),
            "wvh": _pm(wvh_, NE),
            "wvl": _pm(wvl_, NE),
            "woh": _pm(woh_, G),
            "wol": _pm(wol_, G),
            "cosT": cosT,
            "sinT": sinT,
            "rotP": rotP,
            "onesb": onesb,
        })

    res = run_bass_kernel_spmd(_NC, in_maps, list(range(NCORES)))
    LAST_RESULTS = res
    outs = [np.asarray(res.results[i]["out"], dtype=np.float32)
            for i in range(NCORES)]
    full = np.stack([sum(outs[b * G:(b + 1) * G]) for b in range(B)], axis=0)
    return (full / PSC).astype(np.float32)


# revision 29
# speedup vs baseline: 1.0206x; 1.0206x over previous
"""GQA (16 q-heads / 4 kv-heads, D=128, S=2048, E=2048, B=2) on 8 trn2 cores.

Sharding: core = 4*b + g  (b in {0,1} batch, g in {0..3} kv-head group).
Each core computes its batch's 4 query heads (one kv group) end-to-end and
the host sums the 4 partial o_proj outputs per batch.

v4 (balanced Act/PE, paired exp, direct stores):
  - Host pre-arranges all tensors partition-major so every DMA moves >=512B
    contiguous runs (no strided-transpose DMAs); wq/wk/wv/wo resident in
    SBUF fp8 hi/lo planes, x resident per chunk.
  - V projected directly into natural [keys, D] layout (x-tile stationary,
    wv moving) -- no PE transpose, no staging copy.
  - Attention in four 512-wide query chunks; key tiles processed in PAIRS:
    scores for tiles 2j,2j+1 -> one [128,2,512] PSUM tile -> ONE Act exp
    instruction (halves Act instruction overhead); AV per tile in bf16.
    Pipeline depth 2 pairs: scp(j) + drained fill units run before
    avp(j-2), so exp latency is fully hidden.
  - Deferred projection/o_proj work queued as ~2-matmul units and drained a
    few per pair-iteration: B0<-Q1, B1<-Q2+O(c0), B2<-Q3+O(c1), B3<-O(c2),
    tail<-O(c3).  o_proj accumulates in PSUM and stores PSUM->DRAM direct.
"""

import numpy as np
import ml_dtypes

import concourse.bass as bass
import concourse.bacc as bacc
import concourse.mybir as mybir
import concourse.tile as tile
from concourse.bass_utils import run_bass_kernel_spmd

B, S, E = 2, 2048, 2048
H, HKV, D = 16, 4, 128
G = H // HKV          # 4 query heads per kv group
GD = G * D            # 512 channels per group
NCORES = 8
SCALE = 1.0 / float(np.sqrt(D))
ROPE_BASE = 10000.0
AX = 16.0             # fp8 plane scale for x
AW = 64.0             # fp8 plane scale for wq/wk/wv/wo
PSC = AX * AW         # q/k/v come out scaled by PSC
SCALE_EFF = SCALE / (PSC * PSC)   # folds the q*k scale into exp
AO = 16.0             # fp8 plane scale for the normalized attention output
# the softmax reduce uses (PSC/AO)-valued "ones", so ot = AO * attn_out and
# the o_proj result comes out scaled by AO*AW = PSC; the host divides once.
RED = PSC / AO

NE = E // 128         # 16 e-blocks (contraction for projections)
NC4 = S // 512        # 4 position chunks of 512
NST = S // 128        # 16 sk-tiles of 128
NP = NST // 2         # 8 sk-tile PAIRS

F32 = mybir.dt.float32
BF16 = mybir.dt.bfloat16
FP8 = mybir.dt.float8e4
DR = mybir.MatmulPerfMode.DoubleRow
AF = mybir.ActivationFunctionType
OP = mybir.AluOpType

PLANES = ((0, 0), (0, 1), (1, 0))   # (w_plane, x_plane): HH, HL, LH


def _emit(nc, tc, xh, xl, wqh, wql, wkh, wkl, wvh, wvl, woh, wol, cosT,
          sinT, rotP, onesb, out):
    from contextlib import ExitStack
    import collections
    es = ExitStack()
    with es:
        cpool = es.enter_context(tc.tile_pool(name="const", bufs=1))
        xpool = es.enter_context(tc.tile_pool(name="xs", bufs=1))
        rpool = es.enter_context(tc.tile_pool(name="rope", bufs=2))
        etpool = es.enter_context(tc.tile_pool(name="et", bufs=6))
        bcspool = es.enter_context(tc.tile_pool(name="bcs", bufs=2))
        dnpool = es.enter_context(tc.tile_pool(name="dn", bufs=2))
        rcpool = es.enter_context(tc.tile_pool(name="rc", bufs=2))
        otpool = es.enter_context(tc.tile_pool(name="ot", bufs=2))
        ostgpool = es.enter_context(tc.tile_pool(name="ostg", bufs=6))
        pssc = es.enter_context(
            tc.tile_pool(name="pssc", bufs=2, space=bass.MemorySpace.PSUM))
        psav = es.enter_context(
            tc.tile_pool(name="psav", bufs=1, space=bass.MemorySpace.PSUM))
        psmx = es.enter_context(
            tc.tile_pool(name="psmx", bufs=2, space=bass.MemorySpace.PSUM))

        # ---- persistent SBUF tensors ----
        rp_sb = cpool.tile([128, 128], BF16, tag="rp")
        ones_sb = cpool.tile([128, 1], BF16, tag="ones")
        cos_sb = cpool.tile([D, S], BF16, tag="cos")
        sin_sb = cpool.tile([D, S], BF16, tag="sin")
        wk_t = [cpool.tile([128, NE, D], FP8, tag=f"wkt{i}", name=f"wkt{i}")
                for i in range(2)]
        wv_t = [cpool.tile([128, NE, D], FP8, tag=f"wvt{i}", name=f"wvt{i}")
                for i in range(2)]
        wq_t = [cpool.tile([128, NE, GD], FP8, tag=f"wqt{i}", name=f"wqt{i}")
                for i in range(2)]
        wo_t = [cpool.tile([128, G, E], FP8, tag=f"wot{i}", name=f"wot{i}")
                for i in range(2)]
        onescol = cpool.tile([1, 128], BF16, tag="onescol")
        kt = cpool.tile([D, S], BF16, tag="kt")
        qt = [cpool.tile([D, S], BF16, tag=f"qt{h}", name=f"qt{h}")
              for h in range(G)]
        vn = cpool.tile([128, NST, D], BF16, tag="vn")

        xt = {}

        def load_x(c4, nsplit):
            for i, t in enumerate((xh, xl)):
                xtile = xpool.tile([128, NE, 512], FP8, tag=f"x{c4}_{i}",
                                   name=f"x{c4}_{i}")
                step = NE // nsplit
                for s in range(nsplit):
                    nc.sync.dma_start(
                        out=xtile[:, s * step:(s + 1) * step, :],
                        in_=t.ap()[:, s * step:(s + 1) * step,
                                   c4 * 512:(c4 + 1) * 512])
                xt[(c4, i)] = xtile

        # ---- DMA schedule (dependency order; contiguous runs >=512B).
        # Startup interleave: wk halves between x0 quarters so the first
        # K-proj matmuls start as soon as possible.
        nc.sync.dma_start(out=wk_t[0][:, 0:8, :], in_=wkh.ap()[:, 0:8, :])
        xt0h = xpool.tile([128, NE, 512], FP8, tag="x0_0", name="x0_0")
        xt0l = xpool.tile([128, NE, 512], FP8, tag="x0_1", name="x0_1")
        xt[(0, 0)], xt[(0, 1)] = xt0h, xt0l
        nc.sync.dma_start(out=xt0h[:, 0:4, :], in_=xh.ap()[:, 0:4, 0:512])
        nc.sync.dma_start(out=wk_t[0][:, 8:16, :], in_=wkh.ap()[:, 8:16, :])
        nc.sync.dma_start(out=xt0h[:, 4:8, :], in_=xh.ap()[:, 4:8, 0:512])
        nc.sync.dma_start(out=wv_t[0][:], in_=wvh.ap())
        nc.sync.dma_start(out=xt0h[:, 8:12, :], in_=xh.ap()[:, 8:12, 0:512])
        nc.sync.dma_start(out=xt0h[:, 12:16, :], in_=xh.ap()[:, 12:16, 0:512])
        nc.sync.dma_start(out=wv_t[1][:], in_=wvl.ap())
        for sp in range(4):
            nc.sync.dma_start(out=xt0l[:, sp * 4:(sp + 1) * 4, :],
                              in_=xl.ap()[:, sp * 4:(sp + 1) * 4, 0:512])
        nc.sync.dma_start(out=wk_t[1][:], in_=wkl.ap())
        nc.sync.dma_start(out=rp_sb[:], in_=rotP.ap())

        def load_cs(c4):
            sl = slice(c4 * 512, (c4 + 1) * 512)
            nc.sync.dma_start(out=cos_sb[:, sl], in_=cosT.ap()[:, sl])
            nc.sync.dma_start(out=sin_sb[:, sl], in_=sinT.ap()[:, sl])

        load_cs(0)
        nc.sync.dma_start(out=ones_sb[:], in_=onesb.ap())
        nc.vector.memset(onescol[:], 1.0)
        for i, t in enumerate((wqh, wql)):
            for sp in range(2):
                nc.sync.dma_start(
                    out=wq_t[i][:, sp * 8:(sp + 1) * 8, :],
                    in_=t.ap()[:, sp * 8:(sp + 1) * 8, :])
        load_x(1, 2)
        load_cs(1)
        load_x(2, 2)
        load_cs(2)
        load_x(3, 2)
        load_cs(3)
        for i, t in enumerate((woh, wol)):
            for sp in range(2):
                nc.sync.dma_start(
                    out=wo_t[i][:, sp * 2:(sp + 1) * 2, :],
                    in_=t.ap()[:, sp * 2:(sp + 1) * 2, :])

        # ---- fill-unit queues (labelled; require() force-drains FIFO
        # until a label's units are all emitted -- keeps emission order
        # consistent with data dependencies) ----
        fill = collections.deque()
        tailq = collections.deque()
        pending = collections.Counter()

        def enq(label, fn, q=None):
            (fill if q is None else q).append((label, fn))
            pending[label] += 1

        def drain(n, q=None):
            q = fill if q is None else q
            while n > 0 and q:
                lab, fn = q.popleft()
                fn()
                pending[lab] -= 1
                n -= 1

        def require(label):
            while pending.get(label, 0) > 0:
                lab, fn = fill.popleft()
                fn()
                pending[lab] -= 1

        # ---- rope: rotate_half as signed-permutation matmul ----
        def rope_start(ps, eng):
            qraw = rpool.tile([128, 512], BF16, tag="qraw")
            if eng == 'act':
                nc.scalar.copy(qraw[:], ps[:])
            else:
                nc.vector.tensor_copy(qraw[:], ps[:])
            return qraw

        def rope_finish(dst, qraw, sl):
            tmc = rpool.tile([128, 512], BF16, tag="tmc")
            t2 = rpool.tile([128, 512], BF16, tag="t2")
            rot = psmx.tile([128, 512], F32, tag="mx", name="rot")
            nc.tensor.matmul(rot[:], rp_sb[:], qraw[:], start=True, stop=True)
            nc.gpsimd.tensor_tensor(tmc[:], qraw[:], cos_sb[:, sl], OP.mult)
            nc.vector.tensor_tensor(t2[:], rot[:], sin_sb[:, sl], OP.mult)
            nc.vector.tensor_tensor(dst, tmc[:], t2[:], OP.add)

        # ---- projections (fp8 DoubleRow, 3 quant planes) ----
        def proj_mms(wt, cslice, c4):
            mms = []
            for wi, xi in PLANES:
                for p in range(NE // 2):
                    mms.append((wt[wi][:, 2 * p:2 * p + 2, cslice],
                                xt[(c4, xi)][:, 2 * p:2 * p + 2, :]))
            return mms

        def kproj(c4):
            sl = slice(c4 * 512, (c4 + 1) * 512)
            ps = psmx.tile([128, 512], F32, tag="mx", name="ps")
            mms = proj_mms(wk_t, slice(0, D), c4)
            for i, (wa, xa) in enumerate(mms):
                nc.tensor.matmul(ps[:], wa, xa, perf_mode=DR,
                                 start=(i == 0), stop=(i == len(mms) - 1))
            return ps, sl

        def vproj_mms(c4):
            vp = psmx.tile([128, 4, 128], F32, tag="mx", name="vp")
            for i in range(4):
                ksl = slice(i * 128, (i + 1) * 128)
                j = 0
                for wi, xi in PLANES:
                    for p in range(NE // 2):
                        nc.tensor.matmul(
                            vp[:, i, :],
                            xt[(c4, xi)][:, 2 * p:2 * p + 2, ksl],
                            wv_t[wi][:, 2 * p:2 * p + 2, :],
                            perf_mode=DR, start=(j == 0), stop=(j == 23))
                        j += 1
            return vp

        def vn_copy(c4, vp, eng):
            dst = vn[:, c4 * 4:(c4 + 1) * 4, :]
            if eng == 'act':
                nc.scalar.copy(dst, vp[:])
            elif eng == 'pool':
                nc.gpsimd.tensor_copy(dst, vp[:])
            else:
                nc.vector.tensor_copy(dst, vp[:])

        def qproj(h, c4):
            # phase-A inline Q projection; rope finished by caller interleave
            sl = slice(c4 * 512, (c4 + 1) * 512)
            ps = psmx.tile([128, 512], F32, tag="mx", name="ps")
            mms = proj_mms(wq_t, slice(h * D, (h + 1) * D), c4)
            for i, (wa, xa) in enumerate(mms):
                nc.tensor.matmul(ps[:], wa, xa, perf_mode=DR,
                                 start=(i == 0), stop=(i == len(mms) - 1))
            return ps, sl

        def enqueue_qproj(h, c4):
            sl = slice(c4 * 512, (c4 + 1) * 512)
            lab = f"Q{c4}h{h}"
            state = {}
            nmm = 24

            def mk(j):
                def unit():
                    if j == 0:
                        state['ps'] = psmx.tile([128, 512], F32, tag="mx",
                                                name="ps")
                        state['mms'] = proj_mms(
                            wq_t, slice(h * D, (h + 1) * D), c4)
                    ps = state['ps']
                    for jj in (2 * j, 2 * j + 1):
                        wa, xa = state['mms'][jj]
                        nc.tensor.matmul(ps[:], wa, xa, perf_mode=DR,
                                         start=(jj == 0), stop=(jj == nmm - 1))
                return unit
            for j in range(nmm // 2):
                enq(lab, mk(j))

            def fin():
                qraw = rope_start(state['ps'], 'dve')
                drain(2)   # cover the DVE copy latency with queued PE work
                rope_finish(qt[h][:, sl], qraw, sl)
            enq(lab, fin)

        def enqueue_kproj(c4):
            sl = slice(c4 * 512, (c4 + 1) * 512)
            lab = f"K{c4}"
            state = {}
            nmm = 24

            def mk(j):
                def unit():
                    if j == 0:
                        state['ps'] = psmx.tile([128, 512], F32, tag="mx",
                                                name="ps")
                        state['mms'] = proj_mms(wk_t, slice(0, D), c4)
                    ps = state['ps']
                    for jj in (2 * j, 2 * j + 1):
                        wa, xa = state['mms'][jj]
                        nc.tensor.matmul(ps[:], wa, xa, perf_mode=DR,
                                         start=(jj == 0), stop=(jj == nmm - 1))
                return unit
            for j in range(nmm // 2):
                enq(lab, mk(j))

            def fin():
                qraw = rope_start(state['ps'], 'dve')
                drain(2)   # cover the DVE copy latency with queued PE work
                rope_finish(kt[:, sl], qraw, sl)
            enq(lab, fin)

        def enqueue_vproj(c4):
            lab = f"V{c4}"
            state = {}

            def mkmm(i, g):
                def unit():
                    if i == 0 and g == 0:
                        state['vp'] = psmx.tile([128, 4, 128], F32, tag="mx",
                                                name="vp")
                    vp = state['vp']
                    ksl = slice(i * 128, (i + 1) * 128)
                    mms = [(xt[(c4, xi)][:, 2 * p:2 * p + 2, ksl],
                            wv_t[wi][:, 2 * p:2 * p + 2, :])
                           for wi, xi in PLANES for p in range(NE // 2)]
                    for jj in range(8 * g, 8 * g + 8):
                        sa, ma = mms[jj]
                        nc.tensor.matmul(vp[:, i, :], sa, ma, perf_mode=DR,
                                         start=(jj == 0), stop=(jj == 23))
                return unit

            def mkcp(i):
                def unit():
                    nc.vector.tensor_copy(vn[:, c4 * 4 + i, :],
                                          state['vp'][:, i, :])
                return unit
            for i in range(4):
                for g in range(3):
                    enq(lab, mkmm(i, g))
                enq(lab, mkcp(i))

        # ---- o_proj: ot (fp8 hi/lo planes) @ wo, PSUM -> bf16 SBUF
        # staging (two 512-col groups share one [128,1024] staging tile and
        # one store) ----
        def enqueue_oproj(ci, oth, otl, q):
            off = ci * 512
            for st in range(4):
                ssl = slice(st * 128, (st + 1) * 128)
                shared = {}
                for eo in range(4):
                    esl = slice(eo * 512, (eo + 1) * 512)
                    state = {}
                    # head-pair i=0 mms first: they only need heads 0/1 of
                    # the ot planes, so the tail can start before the last
                    # head's epilogue lands.
                    mms = []
                    for i in range(2):
                        for src, wi in ((oth, 0), (oth, 1), (otl, 0)):
                            mms.append((src[:, 2 * i:2 * i + 2, ssl],
                                        wo_t[wi][:, 2 * i:2 * i + 2, esl]))

                    def mk(st, eo, j, mms=mms, state=state, shared=shared):
                        def unit():
                            if j == 0:
                                state['op'] = psmx.tile([128, 512], F32,
                                                        tag="mx", name="op")
                            op = state['op']
                            for jj in (2 * j, 2 * j + 1):
                                oa, wa = mms[jj]
                                nc.tensor.matmul(op[:], oa, wa, perf_mode=DR,
                                                 start=(jj == 0),
                                                 stop=(jj == 5))
                            if j == 2:
                                single = q is tailq and st == 3 and eo >= 2
                                half = eo % 2
                                if half == 0 or single:
                                    shared['ostg'] = ostgpool.tile(
                                        [128, 1024], BF16, tag="ostg",
                                        name="ostg")
                                ostg = shared['ostg']
                                dst = ostg[:, half * 512:(half + 1) * 512]
                                # GPSIMD cannot read PSUM: alternate Act/DVE
                                if (st + eo) % 2 == 0 if single else (
                                        st + eo // 2) % 2 == 0:
                                    nc.scalar.copy(dst, op[:])
                                else:
                                    nc.vector.tensor_copy(dst, op[:])
                                if single:
                                    nc.sync.dma_start(
                                        out=out.ap()[off + st * 128:
                                                     off + (st + 1) * 128,
                                                     eo * 512:
                                                     (eo + 1) * 512],
                                        in_=dst)
                                elif half == 1:
                                    nc.sync.dma_start(
                                        out=out.ap()[off + st * 128:
                                                     off + (st + 1) * 128,
                                                     (eo - 1) * 512:
                                                     (eo + 1) * 512],
                                        in_=ostg[:])
                        return unit
                    for j in range(3):
                        enq(f"O{ci}", mk(st, eo, j), q)

        # ---- K/V as fill units (not used in final schedule; kept simple) --

        # ---- attention: paired key tiles, TWO heads ping-ponged so the
        # sc-PSUM reuse dependency (exp of the same head, 2 pairs back)
        # always has a full other-head iteration of cover; Act runs
        # back-to-back exps and PE never waits on the 2-buffer sc pool ----
        def attn_pass(ci, ha, hb, oth, otl, drain_n):
            off = ci * 512
            qsl = slice(off, off + 512)
            require(f"Q{ci}h{ha}")
            require(f"Q{ci}h{hb}")
            dnA = dnpool.tile([128, 512], BF16, tag="dn", name="dnA")
            dnB = dnpool.tile([128, 512], BF16, tag="dn", name="dnB")
            av2 = psav.tile([128, 2, 512], F32, tag="av")

            def scp_exp(h, j):
                require(f"K{(2 * j + 1) // 4}")
                sc = pssc.tile([128, 2, 512], F32, tag="sc")
                for tt in range(2):
                    t = 2 * j + tt
                    nc.tensor.matmul(sc[:, tt, :],
                                     kt[:, t * 128:(t + 1) * 128],
                                     qt[h][:, qsl], start=True, stop=True)
                et = etpool.tile([128, 2, 512], BF16, tag="et")
                nc.scalar.activation(et[:], sc[:], AF.Exp, scale=SCALE_EFF)
                return et

            def avp(sl_i, j, et):
                require(f"V{(2 * j + 1) // 4}")
                for tt in range(2):
                    t = 2 * j + tt
                    nc.tensor.matmul(av2[:, sl_i, :], vn[:, t, :],
                                     et[:, tt, :],
                                     start=(t == 0), stop=(t == NST - 1))

            def dnp(dn, j, et):
                for tt in range(2):
                    if j == 0 and tt == 0:
                        nc.vector.tensor_copy(dn[:], et[:, 0, :])
                    else:
                        nc.vector.tensor_tensor(dn[:], dn[:], et[:, tt, :],
                                                OP.add)

            def epilogue(h, sl_i, dn):
                drain(6)
                sm = psmx.tile([1, 512], F32, tag="mx", name="sm")
                nc.tensor.matmul(sm[:], ones_sb[:, 0:1], dn[:],
                                 start=True, stop=True)
                rc = rcpool.tile([1, 512], BF16, tag="rc")
                with nc.allow_low_precision(reason="bf16 denom recip"):
                    nc.vector.reciprocal(rc[:], sm[:])
                otf = rpool.tile([D, 512], F32, tag="otf")
                bcs = bcspool.tile([128, 512], BF16, tag="bcs")
                nc.gpsimd.partition_broadcast(bcs[:], rc[:])
                nc.vector.tensor_tensor(otf[:], av2[:, sl_i, :], bcs[:],
                                        OP.mult)
                if h >= 2:
                    nc.scalar.copy(oth[:, h, :], otf[:])
                else:
                    nc.gpsimd.tensor_copy(oth[:, h, :], otf[:])
                nc.vector.tensor_tensor(otl[:, h, :], otf[:], oth[:, h, :],
                                        OP.subtract)

            etsA = {}
            etsB = {}
            etsA[0] = scp_exp(ha, 0)
            etsB[0] = scp_exp(hb, 0)
            etsA[1] = scp_exp(ha, 1)
            drain(drain_n)
            etsB[1] = scp_exp(hb, 1)
            drain(max(drain_n - 1, 1))
            for j in range(2, NP):
                etsA[j] = scp_exp(ha, j)
                drain(drain_n if j % 2 else max(drain_n - 1, 1))
                avp(0, j - 2, etsA[j - 2])
                dnp(dnA, j - 2, etsA[j - 2])
                del etsA[j - 2]
                etsB[j] = scp_exp(hb, j)
                drain(max(drain_n - 1, 1) if j % 2 else drain_n)
                avp(1, j - 2, etsB[j - 2])
                dnp(dnB, j - 2, etsB[j - 2])
                del etsB[j - 2]
            drain(drain_n)
            avp(0, NP - 2, etsA[NP - 2])
            dnp(dnA, NP - 2, etsA[NP - 2])
            avp(1, NP - 2, etsB[NP - 2])
            dnp(dnB, NP - 2, etsB[NP - 2])
            drain(drain_n)
            avp(0, NP - 1, etsA[NP - 1])
            dnp(dnA, NP - 1, etsA[NP - 1])
            epilogue(ha, 0, dnA)
            avp(1, NP - 1, etsB[NP - 1])
            dnp(dnB, NP - 1, etsB[NP - 1])
            epilogue(hb, 1, dnB)

        # ====== phase A: K/V chunks 0-1, Q chunk 0; rest drains into B0 =====
        ps, sl = kproj(0)
        qraw = rope_start(ps, 'act')
        vp = vproj_mms(0)
        rope_finish(kt[:, sl], qraw, sl)
        vn_copy(0, vp, 'act')
        prev = None
        for h in range(G):
            ps, sl = qproj(h, 0)
            if prev is not None:
                ph, pq, psl = prev
                rope_finish(qt[ph][:, psl], pq, psl)
            qraw = rope_start(ps, 'act')
            prev = (h, qraw, sl)
        ps, sl = kproj(1)
        ph, pq, psl = prev
        rope_finish(qt[ph][:, psl], pq, psl)
        qraw = rope_start(ps, 'act')
        vp = vproj_mms(1)
        rope_finish(kt[:, sl], qraw, sl)
        vn_copy(1, vp, 'act')

        # ================= B windows: attention + drained fills =============
        def ot_planes(ci):
            hi = otpool.tile([128, G, 512], FP8, tag="oth", name=f"oth{ci}")
            lo = otpool.tile([128, G, 512], FP8, tag="otl", name=f"otl{ci}")
            return hi, lo

        planes = {}
        # B0: fills = K/V chunks 2,3 (ledger-paced), then Q chunk 1
        planes[0] = ot_planes(0)
        enqueue_kproj(2)
        enqueue_vproj(2)
        enqueue_kproj(3)
        enqueue_vproj(3)
        for hq in range(G):
            enqueue_qproj(hq, 1)
        attn_pass(0, 0, 1, planes[0][0], planes[0][1], 2)
        attn_pass(0, 2, 3, planes[0][0], planes[0][1], 2)
        # B1: fills += Q chunk 2 + o_proj of chunk 0
        planes[1] = ot_planes(1)
        for hq in range(G):
            enqueue_qproj(hq, 2)
        attn_pass(1, 0, 1, planes[1][0], planes[1][1], 2)
        enqueue_oproj(0, planes[0][0], planes[0][1], fill)
        attn_pass(1, 2, 3, planes[1][0], planes[1][1], 2)
        # B2: fills += Q chunk 3 + o_proj of chunk 1
        planes[2] = ot_planes(2)
        for hq in range(G):
            enqueue_qproj(hq, 3)
        attn_pass(2, 0, 1, planes[2][0], planes[2][1], 2)
        enqueue_oproj(1, planes[1][0], planes[1][1], fill)
        attn_pass(2, 2, 3, planes[2][0], planes[2][1], 2)
        # B3: fills += o_proj of chunk 2
        planes[3] = ot_planes(3)
        attn_pass(3, 0, 1, planes[3][0], planes[3][1], 2)
        enqueue_oproj(2, planes[2][0], planes[2][1], fill)
        attn_pass(3, 2, 3, planes[3][0], planes[3][1], 2)
        drain(len(fill))
        # tail: o_proj of chunk 3
        enqueue_oproj(3, planes[3][0], planes[3][1], tailq)
        drain(len(tailq), tailq)


def _build():
    nc = bacc.Bacc("TRN2", target_bir_lowering=False, debug=False,
                   num_devices=NCORES)
    xh = nc.dram_tensor("xh", [128, NE, S], FP8, kind="ExternalInput")
    xl = nc.dram_tensor("xl", [128, NE, S], FP8, kind="ExternalInput")
    wqh = nc.dram_tensor("wqh", [128, NE, GD], FP8, kind="ExternalInput")
    wql = nc.dram_tensor("wql", [128, NE, GD], FP8, kind="ExternalInput")
    wkh = nc.dram_tensor("wkh", [128, NE, D], FP8, kind="ExternalInput")
    wkl = nc.dram_tensor("wkl", [128, NE, D], FP8, kind="ExternalInput")
    wvh = nc.dram_tensor("wvh", [128, NE, D], FP8, kind="ExternalInput")
    wvl = nc.dram_tensor("wvl", [128, NE, D], FP8, kind="ExternalInput")
    woh = nc.dram_tensor("woh", [128, G, E], FP8, kind="ExternalInput")
    wol = nc.dram_tensor("wol", [128, G, E], FP8, kind="ExternalInput")
    cosT = nc.dram_tensor("cosT", [D, S], BF16, kind="ExternalInput")
    sinT = nc.dram_tensor("sinT", [D, S], BF16, kind="ExternalInput")
    rotP = nc.dram_tensor("rotP", [128, 128], BF16, kind="ExternalInput")
    onesb = nc.dram_tensor("onesb", [128, 1], BF16, kind="ExternalInput")
    out = nc.dram_tensor("out", [S, E], BF16, kind="ExternalOutput")
    with tile.TileContext(nc) as tc:
        _emit(nc, tc, xh, xl, wqh, wql, wkh, wkl, wvh, wvl, woh, wol, cosT,
              sinT, rotP, onesb, out)
    nc.compile()
    return nc


def _rope_tables():
    inv = 1.0 / (ROPE_BASE ** (np.arange(0, D, 2, dtype=np.float64) / D))
    t = np.arange(S, dtype=np.float64)
    freqs = t[:, None] * inv[None, :]                    # [S, D/2]
    emb = np.concatenate([freqs, freqs], axis=-1)        # [S, D]
    cosT = np.cos(emb).T.astype(ml_dtypes.bfloat16)      # [D, S]
    sinT = np.sin(emb).T.astype(ml_dtypes.bfloat16)
    return np.ascontiguousarray(cosT), np.ascontiguousarray(sinT)


def _rot_perm():
    # rot(q)[d] = -q[d+64] for d<64, +q[d-64] for d>=64, as a stationary
    # matmul operand: rot = P^T @ q with P[k, m] below.
    p = np.zeros((128, 128), dtype=ml_dtypes.bfloat16)
    for d in range(64):
        p[d + 64, d] = -1.0
        p[d, d + 64] = 1.0
    return p


def _pm(a, nblk):
    """[K, M] -> partition-major [128, nblk, M] (K = nblk*128)."""
    k, m = a.shape
    return np.ascontiguousarray(a.reshape(nblk, 128, m).transpose(1, 0, 2))


_NC = None
LAST_RESULTS = None


def kernel(hidden_states, wq, wk, wv, wo):
    global _NC, LAST_RESULTS
    if _NC is None:
        _NC = _build()
    cosT, sinT = _rope_tables()
    onesb = np.full((128, 1), RED, dtype=ml_dtypes.bfloat16)
    rotP = _rot_perm()
    f8 = ml_dtypes.float8_e4m3

    def planes(a, scale):
        hi = (scale * a).astype(f8)
        lo = (scale * a - hi.astype(np.float32)).astype(f8)
        return hi, lo

    hs = np.asarray(hidden_states, dtype=np.float32)
    wq = np.asarray(wq, dtype=np.float32)
    wk = np.asarray(wk, dtype=np.float32)
    wv = np.asarray(wv, dtype=np.float32)
    wo = np.asarray(wo, dtype=np.float32)
    xplanes = []
    for b in range(B):
        hi, lo = planes(np.ascontiguousarray(hs[b].T), AX)
        xplanes.append((_pm(hi, NE), _pm(lo, NE)))

    in_maps = []
    for core in range(NCORES):
        b, g = divmod(core, G)
        wqh_, wql_ = planes(wq[:, GD * g:GD * (g + 1)], AW)
        wkh_, wkl_ = planes(wk[:, D * g:D * (g + 1)], AW)
        wvh_, wvl_ = planes(wv[:, D * g:D * (g + 1)], AW)
        woh_, wol_ = planes(wo[GD * g:GD * (g + 1), :], AW)
        in_maps.append({
            "xh": xplanes[b][0],
            "xl": xplanes[b][1],
            "wqh": _pm(wqh_, NE),
            "wql": _pm(wql_, NE),
            "wkh": _pm(wkh_, NE),
            "wkl": _pm(wkl_, NE),
            "wvh": _pm(wvh_, NE),
            "wvl": _pm(wvl_, NE),
            "woh": _pm(woh_, G),
            "wol": _pm(wol_, G),
            "cosT": cosT,
            "sinT": sinT,
            "rotP": rotP,
            "onesb": onesb,
        })

    res = run_bass_kernel_spmd(_NC, in_maps, list(range(NCORES)))
    LAST_RESULTS = res
    outs = [np.asarray(res.results[i]["out"], dtype=np.float32)
            for i in range(NCORES)]
    full = np.stack([sum(outs[b * G:(b + 1) * G]) for b in range(B)], axis=0)
    return (full / PSC).astype(np.float32)
